# revision 19
# baseline (speedup 1.0000x reference)
"""CosineTransformerBlock Trainium2 kernel (8 NeuronCores, SPMD, no collectives).

Sharding: core c handles batch b = c // 2 and query-token rows
[ (c % 2) * 1024 : (c % 2) * 1024 + 1024 ] of that batch.  K/V work for a
batch is duplicated across the 2 cores that share it (cheaper than pair
collectives on this chip).

Key algebraic transforms:
  1. Cosine attention has no softmax, so
         (qn @ kn^T) @ v  ==  qn @ (kn^T @ v)
     which turns the O(N^2) attention into two tiny per-head [64,64] matmuls.
  2. The LayerNorm mean-subtraction is folded into the weights on the host:
         LN(x) @ (g*W)  ==  rstd(x) * (x @ Wc),
         Wc = g*W - (1/D) * ones(D) (x) colsum(g*W)
     so the Q/K/V projections consume RAW input tiles (transpose straight
     off the DMA, no LN apply on the critical path).  For the Q and K paths
     the rstd factor cancels inside the per-head l2-normalization, so those
     paths need no LN statistics at all; the V path applies rstd as the
     per-partition scale of its PSUM-evacuation copy.  (Requires the LN
     beta @ W rows to be zero, which holds for this problem's inputs.)

Layout strategy (per core):
  - activations are token-major [tok, feat]; matmul lhsT operands come from
    XBAR DMA-transposes of raw bf16 tiles (cheap, on the SP HWDGE queue);
  - attention-path matmuls are bf16 with fp32 PSUM accumulation;
  - the FFN runs in fp8 e4m3 with MatmulPerfMode.DoubleRow (two packed
    contraction chunks per matmul instruction).  w1 is pre-scaled by W1S and
    w2 by W2S on the host so their ~0.02-sigma weights stay out of the fp8
    denormal range; the scales are divided back out at PSUM evacuation.
    fp8 affects only the FFN branch (~0.5 sigma of a ~2.6 sigma output);
    measured end-to-end rel err is ~1.1e-2 vs the 2e-2 budget.
  - FFN weights are SBUF-resident in fp8 (4 MB each), nothing re-streams;
  - phase 2 runs as [qn prepass for all tiles] -> [attn+wo+LN2 per tile],
    with each tile's FFN LayerNorm (stage A) emitted right after its x so
    the Activation/Pool engines fill the attention-evac latency;
  - all sqrt ops sit in the "sqrt" activation table and every LN2 finishes
    before the first Gelu evacuation, so the Activation engine swaps
    function tables only twice;
  - elementwise work is spread over Activation, DVE and Pool (gpsimd) so no
    single helper engine paces the TensorE stream.
"""

import os
import sys

sys.path.insert(0, "/opt/trn_rl_repo")

import numpy as np
import ml_dtypes

# ---- problem shapes (hardcoded per contract) ----
B, N, D = 4, 2048, 1024
H, DH = 16, 64
INNER = H * DH  # 1024
MLP = 4096
EPS = 1e-5
NCORES = 8
TQ = N // 2  # 1024 query tokens per core
TKV = N  # 2048 kv tokens per core
P = 128
DC = D // P  # 8 chunks of the model dim
IC = INNER // P  # 8
MC = MLP // P  # 32
NQT = TQ // P  # 8 q token tiles
NKT = TKV // P  # 16 kv token tiles
W1S = 32.0  # host-side fp8 scale on w1 (divided out at gelu evac)
W2S = 64.0  # host-side fp8 scale on w2 (divided out at y residual)

BF16 = None  # set lazily (mybir import)
F32 = None
FP8 = None


def _dt():
    global BF16, F32, FP8
    import concourse.mybir as mybir

    BF16 = mybir.dt.bfloat16
    F32 = mybir.dt.float32
    FP8 = mybir.dt.float8e4
    return mybir


def _ln_stats_ops(nc, pool, x_tile, ntok, dfree, eps_tile):
    """bn_stats/bn_aggr over free dim -> (rs, neg_mu_rs) [ntok,1] fp32."""
    import concourse.mybir as mybir

    nsub = (dfree + 511) // 512
    stats = pool.tile([P, nsub, 6], F32, tag="ln_stats")
    xv = x_tile.rearrange("p (s f) -> p s f", s=nsub)
    for s in range(nsub):
        nc.vector.bn_stats(out=stats[:ntok, s, :], in_=xv[:ntok, s, :])
    mv = pool.tile([P, 2], F32, tag="ln_mv")
    nc.vector.bn_aggr(out=mv[:ntok], in_=stats[:ntok])
    rs = pool.tile([P, 1], F32, tag="ln_rs")
    # rs = 1/sqrt(var + eps)
    nc.scalar.activation(
        out=rs[:ntok],
        in_=mv[:ntok, 1:2],
        func=mybir.ActivationFunctionType.Sqrt,
        bias=eps_tile[:ntok],
        scale=1.0,
    )
    nc.vector.reciprocal(out=rs[:ntok], in_=rs[:ntok])
    nmu = pool.tile([P, 1], F32, tag="ln_nmu")
    # nmu = -mu * rs
    nc.vector.tensor_scalar(
        out=nmu[:ntok],
        in0=mv[:ntok, 0:1],
        scalar1=rs[:ntok],
        scalar2=-1.0,
        op0=mybir.AluOpType.mult,
        op1=mybir.AluOpType.mult,
    )
    return rs, nmu


def _var_rstd_ops(nc, pool, x_tile, eps_tile):
    """bn_stats/bn_aggr over free dim -> rstd [P,1] fp32 (no mean needed)."""
    import concourse.mybir as mybir

    stats = pool.tile([P, 2, 6], F32, tag="ln_stats")
    xv = x_tile.rearrange("p (s f) -> p s f", s=2)
    for s in range(2):
        nc.vector.bn_stats(out=stats[:, s, :], in_=xv[:, s, :])
    mv = pool.tile([P, 2], F32, tag="ln_mv")
    nc.vector.bn_aggr(out=mv[:], in_=stats[:])
    rs = pool.tile([P, 1], F32, tag="ln_rs")
    nc.scalar.activation(
        out=rs[:],
        in_=mv[:, 1:2],
        func=mybir.ActivationFunctionType.Sqrt,
        bias=eps_tile[:],
        scale=1.0,
    )
    nc.vector.reciprocal(out=rs[:], in_=rs[:])
    return rs


def build_nc(bias_rows):
    """Build the SPMD program. bias_rows: dict of host-computed fp32 rows.
    The LN-mean weight fold requires bq/bk/bv == 0 (true here: ln1_b == 0);
    bo/b2 are handled via K=1 ones-matmuls when nonzero."""
    assert not np.any(bias_rows["bq"]), "mean-fold requires ln1_b @ wq == 0"
    assert not np.any(bias_rows["bk"]), "mean-fold requires ln1_b @ wk == 0"
    assert not np.any(bias_rows["bv"]), "mean-fold requires ln1_b @ wv == 0"
    mybir = _dt()
    import concourse.tile as tile
    from concourse import bacc

    AF = mybir.ActivationFunctionType
    ALU = mybir.AluOpType
    DR = mybir.MatmulPerfMode.DoubleRow

    nc = bacc.Bacc("TRN2", target_bir_lowering=False, debug=False, num_devices=NCORES)

    # ---- DRAM I/O ----
    Qd = nc.dram_tensor("q_tok", [TQ, D], BF16, kind="ExternalInput").ap()
    Kd = nc.dram_tensor("k_tok", [TKV, D], BF16, kind="ExternalInput").ap()
    Vd = nc.dram_tensor("v_tok", [TKV, D], BF16, kind="ExternalInput").ap()
    wq_d = nc.dram_tensor("wq", [D, INNER], BF16, kind="ExternalInput").ap()
    wk_d = nc.dram_tensor("wk", [D, INNER], BF16, kind="ExternalInput").ap()
    wv_d = nc.dram_tensor("wv", [D, INNER], BF16, kind="ExternalInput").ap()
    wo_d = nc.dram_tensor("wo", [INNER, D], BF16, kind="ExternalInput").ap()
    # fp8 FFN weights, packed on the host for DoubleRow lhsT/rhs slicing:
    #   w1p[p, m, e, q] = W1S * w1c[e*128 + p, m*128 + q]
    #   w2p[p, e, n]    = W2S * w2[e*128 + p, n]
    w1_d = nc.dram_tensor("w1p", [P, MC * DC * P], FP8, kind="ExternalInput").ap()
    w2_d = nc.dram_tensor("w2p", [P, MC * D], FP8, kind="ExternalInput").ap()
    bff1_d = nc.dram_tensor("bff1", [P, MC], F32, kind="ExternalInput").ap()
    brow_d = {}
    for name in ("bo", "b2"):
        if np.any(bias_rows[name]):
            brow_d[name] = nc.dram_tensor(
                "brow_" + name, [1, bias_rows[name].shape[0]], BF16,
                kind="ExternalInput",
            ).ap()
    Yd = nc.dram_tensor("y", [TQ, D], BF16, kind="ExternalOutput").ap()

    Qt = Qd.rearrange("(t p) d -> t p d", p=P)
    Kt = Kd.rearrange("(t p) d -> t p d", p=P)
    Vt = Vd.rearrange("(t p) d -> t p d", p=P)
    Yt = Yd.rearrange("(t p) d -> t p d", p=P)
    # weight DRAM views: [P, chunk, cols]
    wq_v = wq_d.rearrange("(c p) n -> p c n", p=P)
    wk_v = wk_d.rearrange("(c p) n -> p c n", p=P)
    wv_v = wv_d.rearrange("(c p) n -> p c n", p=P)
    wo_v = wo_d.rearrange("(c p) n -> p c n", p=P)
    w1_v = w1_d.rearrange("p (m e q) -> p m e q", m=MC, e=DC)  # [P,MC,8,P]
    w2_v = w2_d.rearrange("p (e n) -> p e n", e=MC)

    with tile.TileContext(nc) as tc:
        with tc.tile_pool(name="singles", bufs=1) as singles:
            # persistent weights (wk/wv live in a scoped pool around phase 1
            # and their SBUF is recycled for the phase-2 pools)
            wq_sb = singles.tile([P, DC, INNER], BF16)
            wo_sb = singles.tile([P, IC, D], BF16)
            w1_sb = singles.tile([P, MC, DC, P], FP8)
            w2_sb = singles.tile([P, MC, D], FP8)
            bff1_sb = singles.tile([P, MC], F32)
            eps_tile = singles.tile([P, 1], F32)
            nc.vector.memset(eps_tile[:], EPS)
            ones_row = singles.tile([1, P], BF16)
            nc.vector.memset(ones_row[:], 1.0)
            brow_sb = {}
            for name, ap in brow_d.items():
                t = singles.tile([1, ap.shape[1]], BF16, tag="brow_" + name)
                nc.gpsimd.dma_start(t[:], ap[:])
                brow_sb[name] = t
            # residual / LN2 source (bf16: feeds LN stats + final residual)
            x_sb = singles.tile([P, NQT, D], BF16)
            # head-pair attention matrices: M_sb[:, pr, :] is
            # blockdiag(M_2pr, M_2pr+1); off-diagonal junk stays zero
            M_sb = singles.tile([P, IC, P], BF16)
            nc.vector.memset(M_sb[:], 0.0)
            # feature-major fp8 LN2(x) for the whole 1024-token slab
            xnT8_sb = singles.tile([P, DC, TQ], FP8)

            # ---------------- Phase 1: K/V -> M_h ----------------
            with (
                tc.tile_pool(name="kvw", bufs=1) as kvw,
                tc.tile_pool(name="kv_io", bufs=6) as kv_io,
                tc.tile_pool(name="kv_mid", bufs=3) as kv_mid,
                tc.tile_pool(name="kv_stats", bufs=4) as kv_stats,
                tc.tile_pool(name="kv_ps", bufs=6, space="PSUM") as kv_ps,
                tc.tile_pool(name="m_ps", bufs=1, space="PSUM") as m_ps_pool,
            ):
                # wk/wv first on the Pool queue (chunked into separate tiles:
                # tile 0's projection starts after one chunk and later chunk
                # writes can't alias earlier reads), then later-phase weights.
                wk_sb = [kvw.tile([P, INNER], BF16, name=f"wk{c}") for c in range(DC)]
                wv_sb = [kvw.tile([P, INNER], BF16, name=f"wv{c}") for c in range(DC)]
                for c in range(DC):
                    nc.gpsimd.dma_start(wk_sb[c][:], wk_v[:, c, :])
                for c in range(DC):
                    nc.gpsimd.dma_start(wv_sb[c][:], wv_v[:, c, :])
                nc.gpsimd.dma_start(wq_sb[:], wq_v[:])
                nc.gpsimd.dma_start(wo_sb[:], wo_v[:])
                for mg in range(4):
                    nc.gpsimd.dma_start(
                        w1_sb[:, mg * 8 : (mg + 1) * 8], w1_v[:, mg * 8 : (mg + 1) * 8]
                    )
                for mg in range(4):
                    nc.gpsimd.dma_start(
                        w2_sb[:, mg * 8 : (mg + 1) * 8], w2_v[:, mg * 8 : (mg + 1) * 8]
                    )
                nc.gpsimd.dma_start(bff1_sb[:], bff1_d[:])
                M_ps = m_ps_pool.tile([P, IC, P], F32)

                def m_accum(t, kn_bf, v_bf):
                    # M_h accumulation: M[h] += kn_h^T @ v_h
                    # Heads are processed in pairs: one [128,128] matmul per
                    # pair computes blockdiag(M_2pr, M_2pr+1) plus junk
                    # off-diagonal blocks (discarded at evac). start=True
                    # zeroes the whole 2KB PSUM zero-region (= 4 pair blocks),
                    # so only the first pair per region starts the group and
                    # only the last stops it.  Called one tile late (software
                    # pipelining) so the kn/v evac chains are never on the
                    # TensorE critical path.
                    kn_flat = kn_bf.rearrange("p h f -> p (h f)")
                    for pr in range(IC):
                        nc.tensor.matmul(
                            M_ps[:, pr, :],
                            kn_flat[:, pr * P : (pr + 1) * P],
                            v_bf[:, pr * P : (pr + 1) * P],
                            start=(t == 0 and pr % 4 == 0),
                            stop=(t == NKT - 1 and pr % 4 == 3),
                            skip_group_check=True,
                        )

                prev_kv = None
                for t in range(NKT):
                    kn_bf = None
                    v_bf = None
                    for which in ("k", "v"):
                        src = Kt[t] if which == "k" else Vt[t]
                        w_sb = wk_sb if which == "k" else wv_sb
                        x_in = kv_io.tile([P, D], BF16, tag="kv_in")
                        nc.scalar.dma_start(x_in[:], src[:])
                        # transpose RAW tile; LN mean is folded into Wc and
                        # rstd either cancels (K) or scales the evac (V)
                        xnT = kv_mid.tile([P, DC, P], BF16, tag="kv_xnT")
                        for c in range(DC):
                            nc.sync.dma_start(
                                xnT[:, c, :], x_in[:, c * P : (c + 1) * P],
                                transpose=True,
                            )
                        if which == "v":
                            rs_v = _var_rstd_ops(nc, kv_stats, x_in, eps_tile)
                        pss = []
                        for g in range(2):
                            ps = kv_ps.tile([P, 512], F32, tag="kv_proj")
                            pss.append(ps)
                        for c in range(DC):
                            for g in range(2):
                                nc.tensor.matmul(
                                    pss[g][:],
                                    xnT[:, c, :],
                                    w_sb[c][:, g * 512 : (g + 1) * 512],
                                    start=(c == 0),
                                    stop=(c == DC - 1),
                                )
                        if which == "v":
                            # v = rstd * (x @ wv_c)
                            v_bf = kv_mid.tile([P, INNER], BF16, tag="v_bf")
                            for g in range(2):
                                nc.scalar.activation(
                                    out=v_bf[:, g * 512 : (g + 1) * 512],
                                    in_=pss[g][:],
                                    func=AF.Copy,
                                    scale=rs_v[:],
                                )
                        else:
                            # l2-normalize per head (rstd cancels)
                            kn_bf = kv_mid.tile([P, H, DH], BF16, tag="kn_bf")
                            for g in range(2):
                                sq = kv_mid.tile([P, 512], F32, tag="kv_sq")
                                nc.scalar.activation(
                                    out=sq[:], in_=pss[g][:], func=AF.Square
                                )
                                ss = kv_stats.tile([P, 8, 1], F32, tag="l2_ss")
                                nc.vector.reduce_sum(
                                    out=ss[:],
                                    in_=sq.rearrange("p (h f) -> p h f", h=8),
                                    axis=mybir.AxisListType.X,
                                )
                                rn = kv_stats.tile([P, 8, 1], F32, tag="l2_rn")
                                nc.scalar.activation(
                                    out=rn[:], in_=ss[:], func=AF.Sqrt
                                )
                                nc.vector.tensor_scalar_max(
                                    out=rn[:], in0=rn[:], scalar1=1e-12
                                )
                                nc.vector.reciprocal(out=rn[:], in_=rn[:])
                                nc.vector.tensor_tensor(
                                    out=kn_bf[:, g * 8 : (g + 1) * 8, :],
                                    in0=pss[g].rearrange("p (h f) -> p h f", h=8),
                                    in1=rn.to_broadcast([P, 8, DH]),
                                    op=ALU.mult,
                                )
                    if prev_kv is not None:
                        m_accum(t - 1, *prev_kv)
                    prev_kv = (kn_bf, v_bf)
                m_accum(NKT - 1, *prev_kv)
                for po in (0, 64):
                    nc.scalar.activation(
                        out=M_sb[po : po + 64, :, po : po + 64],
                        in_=M_ps[po : po + 64, :, po : po + 64],
                        func=AF.Copy,
                    )

            # ---------------- Phase 2: Q -> attn -> x -> LN2 ----------------
            with (
                tc.tile_pool(name="q_io", bufs=NQT) as q_io,
                tc.tile_pool(name="q_mid", bufs=2) as q_mid,
                tc.tile_pool(name="q_qn", bufs=NQT) as q_qn,
                tc.tile_pool(name="q_at", bufs=NQT) as q_at,
                tc.tile_pool(name="q_stats", bufs=4) as q_stats,
                tc.tile_pool(name="f_mid", bufs=2) as f_mid,
                tc.tile_pool(name="f_stats", bufs=4) as f_stats,
                tc.tile_pool(name="q_ps", bufs=2, space="PSUM") as q_ps,
                tc.tile_pool(name="x_ps", bufs=2, space="PSUM") as x_ps,
                tc.tile_pool(name="at_ps", bufs=2, space="PSUM") as at_ps,
            ):
                # --- qn prepass: all tiles ---
                q_ins = []
                qnT2s = []
                for t in range(NQT):
                    q_in = q_io.tile([P, D], BF16, tag="q_in", name=f"q_in{t}")
                    nc.sync.dma_start(q_in[:], Qt[t][:])
                    q_ins.append(q_in)
                    qnT = q_mid.tile([P, DC, P], BF16, tag="q_xnT")
                    for c in range(DC):
                        nc.sync.dma_start(
                            qnT[:, c, :], q_in[:, c * P : (c + 1) * P], transpose=True
                        )
                    pss = []
                    for g in range(2):
                        ps = q_ps.tile([P, 512], F32, tag="q_proj")
                        pss.append(ps)
                    for c in range(DC):
                        for g in range(2):
                            nc.tensor.matmul(
                                pss[g][:],
                                qnT[:, c, :],
                                wq_sb[:, c, g * 512 : (g + 1) * 512],
                                start=(c == 0),
                                stop=(c == DC - 1),
                            )
                    # l2-normalize q per head (rstd cancels) -> qn bf16
                    qn_bf = q_mid.tile([P, H, DH], BF16, tag="qn_bf")
                    for g in range(2):
                        sq = q_mid.tile([P, 512], F32, tag="q_sq")
                        nc.scalar.activation(out=sq[:], in_=pss[g][:], func=AF.Square)
                        ss = q_stats.tile([P, 8, 1], F32, tag="ql2_ss")
                        nc.vector.reduce_sum(
                            out=ss[:],
                            in_=sq.rearrange("p (h f) -> p h f", h=8),
                            axis=mybir.AxisListType.X,
                        )
                        rn = q_stats.tile([P, 8, 1], F32, tag="ql2_rn")
                        nc.scalar.activation(out=rn[:], in_=ss[:], func=AF.Sqrt)
                        nc.vector.tensor_scalar_max(out=rn[:], in0=rn[:], scalar1=1e-12)
                        nc.vector.reciprocal(out=rn[:], in_=rn[:])
                        nc.vector.tensor_tensor(
                            out=qn_bf[:, g * 8 : (g + 1) * 8, :],
                            in0=pss[g].rearrange("p (h f) -> p h f", h=8),
                            in1=rn.to_broadcast([P, 8, DH]),
                            op=ALU.mult,
                        )
                    # transpose qn -> [INNER, tok] feature-major
                    qn_flat = qn_bf.rearrange("p h f -> p (h f)")
                    qnT2 = q_qn.tile([P, IC, P], BF16, tag="qnT2", name=f"qnT2_{t}")
                    for c in range(IC):
                        nc.sync.dma_start(
                            qnT2[:, c, :], qn_flat[:, c * P : (c + 1) * P],
                            transpose=True,
                        )
                    qnT2s.append(qnT2)
                # --- attn + wo + residual + LN2(FFN stage A) per tile ---
                aT_bfs = []
                for t in range(NQT):
                    # attn^T[h] = M_h^T @ qn_h^T  -> [INNER, tok] chunks
                    a_ps = at_ps.tile([P, IC, P], F32, tag="attn_ps")
                    for pr in range(IC):
                        nc.tensor.matmul(
                            a_ps[:, pr, :],
                            M_sb[:, pr, :],
                            qnT2s[t][:, pr, :],
                            start=True,
                            stop=True,
                            skip_group_check=True,
                        )
                    aT_bf = q_at.tile([P, IC, P], BF16, tag="aT_bf")
                    nc.scalar.activation(out=aT_bf[:], in_=a_ps[:], func=AF.Copy)
                    aT_bfs.append(aT_bf)
                for t in range(NQT):
                    aT_bf = aT_bfs[t]
                    # x = Q + attn @ wo (+bo)
                    xps = []
                    for g in range(2):
                        ps = x_ps.tile([P, 512], F32, tag="x_proj")
                        xps.append(ps)
                    for c in range(IC):
                        for g in range(2):
                            nc.tensor.matmul(
                                xps[g][:],
                                aT_bf[:, c, :],
                                wo_sb[:, c, g * 512 : (g + 1) * 512],
                                start=(c == 0),
                                stop=(c == IC - 1) and ("bo" not in brow_sb),
                            )
                    for g in range(2):
                        if "bo" in brow_sb:
                            nc.tensor.matmul(
                                xps[g][:],
                                ones_row[:],
                                brow_sb["bo"][:, g * 512 : (g + 1) * 512],
                                start=False,
                                stop=True,
                                skip_group_check=True,
                            )
                        # x = 1.0*xps + q_in  (fused DVE op; PSUM in0)
                        nc.vector.scalar_tensor_tensor(
                            out=x_sb[:, t, g * 512 : (g + 1) * 512],
                            in0=xps[g][:],
                            scalar=1.0,
                            in1=q_ins[t][:, g * 512 : (g + 1) * 512],
                            op0=ALU.mult,
                            op1=ALU.add,
                        )
                    # FFN stage A for this tile: LN2 -> transpose -> fp8 cast.
                    # Emitted here so it runs in the attention-phase slack.
                    rs, nmu = _ln_stats_ops(nc, f_stats, x_sb[:, t, :], P, D, eps_tile)
                    xn = f_mid.tile([P, D], BF16, tag="f_std")
                    nc.scalar.activation(
                        out=xn[:],
                        in_=x_sb[:, t, :],
                        func=AF.Identity,
                        bias=nmu[:],
                        scale=rs[:],
                    )
                    xnT = f_mid.tile([P, DC, P], BF16, tag="f_xnT")
                    for c in range(DC):
                        nc.sync.dma_start(
                            xnT[:, c, :], xn[:, c * P : (c + 1) * P], transpose=True
                        )
                    # cast the tile's transposed slab to fp8 (Pool engine)
                    nc.gpsimd.tensor_copy(
                        out=xnT8_sb[:, :, t * P : (t + 1) * P],
                        in_=xnT[:],
                    )

            # ---------------- Phase 3: FFN (fp8 DoubleRow) ----------------
            # h^T = gelu(w1^T xn^T) in fp8, then y = x + h @ w2 / W2S
            # token-major.  DoubleRow packs two 128-deep chunks per matmul.
            with (
                tc.tile_pool(name="f_h", bufs=1) as f_h,
                tc.tile_pool(name="f_out", bufs=3) as f_out,
                tc.tile_pool(name="h_ps", bufs=2, space="PSUM") as h_ps,
                tc.tile_pool(name="y_ps", bufs=4, space="PSUM") as y_ps,
            ):
                GT = 8  # token tiles per group (single group: the w2 matmuls
                # drain while the Activation engine finishes the gelu backlog)
                for g3 in range(NQT // GT):
                    tok0 = g3 * GT * P
                    h8 = f_h.tile([P, MC, GT * P], FP8, tag="h8")
                    for m in range(MC):
                        # one matmul per (contraction pair, 512-col half):
                        # a single matmul output may not exceed one PSUM bank
                        hp = h_ps.tile([P, GT * P // 512, 512], F32, tag="h_ps_t")
                        for j in range(DC // 2):
                            for g4 in range(GT * P // 512):
                                nc.tensor.matmul(
                                    hp[:, g4, :],
                                    w1_sb[:, m, 2 * j : 2 * j + 2, :],
                                    xnT8_sb[:, 2 * j : 2 * j + 2,
                                            tok0 + g4 * 512 : tok0 + (g4 + 1) * 512],
                                    start=(j == 0),
                                    stop=(j == DC // 2 - 1),
                                    perf_mode=DR,
                                )
                        # h = gelu(psum / W1S + b1row), straight to fp8
                        nc.scalar.activation(
                            out=h8[:, m, :],
                            in_=hp.rearrange("p a b -> p (a b)"),
                            func=AF.Gelu,
                            bias=bff1_sb[:, m : m + 1],
                            scale=1.0 / W1S,
                        )
                    for tt in range(GT):
                        t = g3 * GT + tt
                        for g in range(2):
                            yps = y_ps.tile([P, 512], F32, tag="y_ps_t")
                            for j2 in range(MC // 2):
                                nc.tensor.matmul(
                                    yps[:],
                                    h8[:, 2 * j2 : 2 * j2 + 2, tt * P : (tt + 1) * P],
                                    w2_sb[:, 2 * j2 : 2 * j2 + 2, g * 512 : (g + 1) * 512],
                                    start=(j2 == 0),
                                    stop=(j2 == MC // 2 - 1) and ("b2" not in brow_sb),
                                    perf_mode=DR,
                                )
                            if "b2" in brow_sb:
                                nc.tensor.matmul(
                                    yps[:],
                                    ones_row[:],
                                    brow_sb["b2"][:, g * 512 : (g + 1) * 512],
                                    start=False,
                                    stop=True,
                                    skip_group_check=True,
                                )
                            y_out = f_out.tile([P, 512], BF16, tag="y_out")
                            # y = yps / W2S + x  (single fused DVE op)
                            nc.vector.scalar_tensor_tensor(
                                out=y_out[:],
                                in0=yps[:],
                                scalar=1.0 / W2S,
                                in1=x_sb[:, t, g * 512 : (g + 1) * 512],
                                op0=ALU.mult,
                                op1=ALU.add,
                            )
                            nc.sync.dma_start(
                                Yt[t][:, g * 512 : (g + 1) * 512], y_out[:]
                            )

    nc.compile()
    return nc


def prep_inputs(inputs):
    """Host-side shard + weight folding. Returns (in_maps, bias_rows)."""
    f32 = np.float32
    bf = ml_dtypes.bfloat16
    f8 = ml_dtypes.float8_e4m3fn
    g1 = np.asarray(inputs["ln1_g"], f32)
    b1ln = np.asarray(inputs["ln1_b"], f32)
    g2 = np.asarray(inputs["ln2_g"], f32)
    b2ln = np.asarray(inputs["ln2_b"], f32)
    wq = np.asarray(inputs["wq"], f32)
    wk = np.asarray(inputs["wk"], f32)
    wv = np.asarray(inputs["wv"], f32)
    wo = np.asarray(inputs["wo"], f32)
    w1 = np.asarray(inputs["w1"], f32)
    w2 = np.asarray(inputs["w2"], f32)

    bias_rows = {
        "bq": (b1ln @ wq).astype(f32),
        "bk": (b1ln @ wk).astype(f32),
        "bv": (b1ln @ wv).astype(f32),
        "bo": np.asarray(inputs["bo"], f32),
        # b2 joins the W2S-scaled yps accumulation group
        "b2": (np.asarray(inputs["b2"], f32) * W2S),
    }
    bff1 = (b2ln @ w1 + np.asarray(inputs["b1"], f32)).astype(f32)
    bff1_tile = np.ascontiguousarray(bff1.reshape(MC, P).T)  # [P, MC]

    def mean_fold(Wg):
        # LN(x) @ Wg == rstd(x) * (x @ Wc):  Wc = Wg - (1/D) * 1 (x) colsum(Wg)
        return Wg - Wg.sum(axis=0, keepdims=True) / Wg.shape[0]

    wq_b = np.ascontiguousarray(mean_fold(g1[:, None] * wq).astype(bf))
    wk_b = np.ascontiguousarray(mean_fold(g1[:, None] * wk).astype(bf))
    wv_b = np.ascontiguousarray(mean_fold(g1[:, None] * wv).astype(bf))
    wo_b = np.ascontiguousarray(wo.astype(bf))
    # fp8 packs (see build_nc for index meaning); LN2's mean stays on-chip
    # (the xn path) so w1 keeps only the gamma fold.
    w1g = (g2[:, None] * w1) * W1S
    w1_b = np.ascontiguousarray(
        w1g.reshape(DC, P, MC, P).transpose(1, 2, 0, 3).reshape(P, MC * DC * P)
    ).astype(f8)
    w2s = w2 * W2S
    w2_b = np.ascontiguousarray(
        w2s.reshape(MC, P, D).transpose(1, 0, 2).reshape(P, MC * D)
    ).astype(f8)

    Q = np.asarray(inputs["Q"], f32)
    K = np.asarray(inputs["K"], f32)
    V = np.asarray(inputs["V"], f32)

    in_maps = []
    for c in range(NCORES):
        b = c // 2
        r0 = (c % 2) * TQ
        m = {
            "q_tok": np.ascontiguousarray(Q[b, r0 : r0 + TQ].astype(bf)),
            "k_tok": np.ascontiguousarray(K[b].astype(bf)),
            "v_tok": np.ascontiguousarray(V[b].astype(bf)),
            "wq": wq_b,
            "wk": wk_b,
            "wv": wv_b,
            "wo": wo_b,
            "w1p": w1_b,
            "w2p": w2_b,
            "bff1": bff1_tile,
        }
        for name, row in bias_rows.items():
            if name in ("bo", "b2") and np.any(row):
                m["brow_" + name] = row[None, :].astype(bf)
        in_maps.append(m)
    return in_maps, bias_rows


_NC_CACHE = {}


def kernel(**inputs) -> np.ndarray:
    from concourse.bass_utils import run_bass_kernel_spmd

    in_maps, bias_rows = prep_inputs(inputs)
    bias_key = tuple(sorted(n for n, r in bias_rows.items() if np.any(r)))
    if bias_key not in _NC_CACHE:
        _NC_CACHE[bias_key] = build_nc(bias_rows)
    nc = _NC_CACHE[bias_key]
    res = run_bass_kernel_spmd(nc, in_maps, core_ids=list(range(NCORES)))
    out = np.empty((B, N, D), np.float32)
    for c in range(NCORES):
        b = c // 2
        r0 = (c % 2) * TQ
        out[b, r0 : r0 + TQ] = res.results[c]["y"]
    return out


# revision 20
# speedup vs baseline: 1.1032x; 1.1032x over previous
"""CosineTransformerBlock Trainium2 kernel (8 NeuronCores, SPMD, no collectives).

Sharding: core c handles batch b = c // 2 and query-token rows
[ (c % 2) * 1024 : (c % 2) * 1024 + 1024 ] of that batch.  K/V work for a
batch is duplicated across the 2 cores that share it (cheaper than pair
collectives on this chip).

Key algebraic transforms:
  1. Cosine attention has no softmax, so
         (qn @ kn^T) @ v  ==  qn @ (kn^T @ v)
     which turns the O(N^2) attention into two tiny per-head [64,64] matmuls.
  2. The LayerNorm mean-subtraction is folded into the weights on the host:
         LN(x) @ (g*W)  ==  rstd(x) * (x @ Wc),
         Wc = g*W - (1/D) * ones(D) (x) colsum(g*W)
     so the Q/K/V projections consume RAW input tiles (transpose straight
     off the DMA, no LN apply on the critical path).  For the Q and K paths
     the rstd factor cancels inside the per-head l2-normalization, so those
     paths need no LN statistics at all; the V path applies rstd as the
     per-partition scale of its PSUM-evacuation copy.  (Requires the LN
     beta @ W rows to be zero, which holds for this problem's inputs.)

Layout strategy (per core):
  - activations are token-major [tok, feat]; matmul lhsT operands come from
    XBAR DMA-transposes of raw bf16 tiles (cheap, on the SP HWDGE queue);
  - attention-path matmuls are bf16 with fp32 PSUM accumulation;
  - the FFN runs in fp8 e4m3 with MatmulPerfMode.DoubleRow (two packed
    contraction chunks per matmul instruction).  w1 is pre-scaled by W1S and
    w2 by W2S on the host so their ~0.02-sigma weights stay out of the fp8
    denormal range; the scales are divided back out at PSUM evacuation.
    fp8 affects only the FFN branch (~0.5 sigma of a ~2.6 sigma output);
    measured end-to-end rel err is ~1.1e-2 vs the 2e-2 budget.
  - FFN weights are SBUF-resident in fp8 (4 MB each), nothing re-streams;
  - phase 2 runs as [qn prepass for all tiles] -> [attn+wo+LN2 per tile],
    with each tile's FFN LayerNorm (stage A) emitted right after its x so
    the Activation/Pool engines fill the attention-evac latency;
  - all sqrt ops sit in the "sqrt" activation table and every LN2 finishes
    before the first Gelu evacuation, so the Activation engine swaps
    function tables only twice;
  - elementwise work is spread over Activation, DVE and Pool (gpsimd) so no
    single helper engine paces the TensorE stream.
"""

import os
import sys

sys.path.insert(0, "/opt/trn_rl_repo")

import numpy as np
import ml_dtypes

# ---- problem shapes (hardcoded per contract) ----
B, N, D = 4, 2048, 1024
H, DH = 16, 64
INNER = H * DH  # 1024
MLP = 4096
EPS = 1e-5
NCORES = 8
TQ = N // 2  # 1024 query tokens per core
TKV = N  # 2048 kv tokens per core
P = 128
DC = D // P  # 8 chunks of the model dim
IC = INNER // P  # 8
MC = MLP // P  # 32
NQT = TQ // P  # 8 q token tiles
NKT = TKV // P  # 16 kv token tiles
W1S = 32.0  # host-side fp8 scale on w1 (divided out at gelu evac)
W2S = 64.0  # host-side fp8 scale on w2 (divided out at y residual)

BF16 = None  # set lazily (mybir import)
F32 = None
FP8 = None


def _dt():
    global BF16, F32, FP8
    import concourse.mybir as mybir

    BF16 = mybir.dt.bfloat16
    F32 = mybir.dt.float32
    FP8 = mybir.dt.float8e4
    return mybir


def _ln_stats_ops(nc, pool, x_tile, ntok, dfree, eps_tile):
    """bn_stats/bn_aggr over free dim -> (rs, neg_mu_rs) [ntok,1] fp32."""
    import concourse.mybir as mybir

    nsub = (dfree + 511) // 512
    stats = pool.tile([P, nsub, 6], F32, tag="ln_stats")
    xv = x_tile.rearrange("p (s f) -> p s f", s=nsub)
    for s in range(nsub):
        nc.vector.bn_stats(out=stats[:ntok, s, :], in_=xv[:ntok, s, :])
    mv = pool.tile([P, 2], F32, tag="ln_mv")
    nc.vector.bn_aggr(out=mv[:ntok], in_=stats[:ntok])
    rs = pool.tile([P, 1], F32, tag="ln_rs")
    # rs = 1/sqrt(var + eps)
    nc.scalar.activation(
        out=rs[:ntok],
        in_=mv[:ntok, 1:2],
        func=mybir.ActivationFunctionType.Sqrt,
        bias=eps_tile[:ntok],
        scale=1.0,
    )
    nc.vector.reciprocal(out=rs[:ntok], in_=rs[:ntok])
    nmu = pool.tile([P, 1], F32, tag="ln_nmu")
    # nmu = -mu * rs
    nc.vector.tensor_scalar(
        out=nmu[:ntok],
        in0=mv[:ntok, 0:1],
        scalar1=rs[:ntok],
        scalar2=-1.0,
        op0=mybir.AluOpType.mult,
        op1=mybir.AluOpType.mult,
    )
    return rs, nmu


def _var_rstd_ops(nc, pool, x_tile, eps_tile):
    """bn_stats/bn_aggr over free dim -> rstd [P,1] fp32 (no mean needed)."""
    import concourse.mybir as mybir

    stats = pool.tile([P, 2, 6], F32, tag="ln_stats")
    xv = x_tile.rearrange("p (s f) -> p s f", s=2)
    for s in range(2):
        nc.vector.bn_stats(out=stats[:, s, :], in_=xv[:, s, :])
    mv = pool.tile([P, 2], F32, tag="ln_mv")
    nc.vector.bn_aggr(out=mv[:], in_=stats[:])
    rs = pool.tile([P, 1], F32, tag="ln_rs")
    nc.scalar.activation(
        out=rs[:],
        in_=mv[:, 1:2],
        func=mybir.ActivationFunctionType.Sqrt,
        bias=eps_tile[:],
        scale=1.0,
    )
    nc.vector.reciprocal(out=rs[:], in_=rs[:])
    return rs


def build_nc(bias_rows):
    """Build the SPMD program. bias_rows: dict of host-computed fp32 rows.
    The LN-mean weight fold requires bq/bk/bv == 0 (true here: ln1_b == 0);
    bo/b2 are handled via K=1 ones-matmuls when nonzero."""
    assert not np.any(bias_rows["bq"]), "mean-fold requires ln1_b @ wq == 0"
    assert not np.any(bias_rows["bk"]), "mean-fold requires ln1_b @ wk == 0"
    assert not np.any(bias_rows["bv"]), "mean-fold requires ln1_b @ wv == 0"
    mybir = _dt()
    import concourse.tile as tile
    from concourse import bacc

    AF = mybir.ActivationFunctionType
    ALU = mybir.AluOpType
    DR = mybir.MatmulPerfMode.DoubleRow

    nc = bacc.Bacc("TRN2", target_bir_lowering=False, debug=False, num_devices=NCORES)

    # ---- DRAM I/O ----
    Qd = nc.dram_tensor("q_tok", [TQ, D], BF16, kind="ExternalInput").ap()
    Kd = nc.dram_tensor("k_tok", [TKV, D], BF16, kind="ExternalInput").ap()
    Vd = nc.dram_tensor("v_tok", [TKV, D], BF16, kind="ExternalInput").ap()
    wq_d = nc.dram_tensor("wq", [D, INNER], BF16, kind="ExternalInput").ap()
    wk_d = nc.dram_tensor("wk", [D, INNER], BF16, kind="ExternalInput").ap()
    wv_d = nc.dram_tensor("wv", [D, INNER], BF16, kind="ExternalInput").ap()
    wo_d = nc.dram_tensor("wo", [INNER, D], BF16, kind="ExternalInput").ap()
    # fp8 FFN weights, packed on the host for DoubleRow lhsT/rhs slicing:
    #   w1p[p, m, e, q] = W1S * w1c[e*128 + p, m*128 + q]
    #   w2p[p, e, n]    = W2S * w2[e*128 + p, n]
    w1_d = nc.dram_tensor("w1p", [P, MC * DC * P], FP8, kind="ExternalInput").ap()
    w2_d = nc.dram_tensor("w2p", [P, MC * D], FP8, kind="ExternalInput").ap()
    bff1_d = nc.dram_tensor("bff1", [P, MC], F32, kind="ExternalInput").ap()
    brow_d = {}
    for name in ("bo", "b2"):
        if np.any(bias_rows[name]):
            brow_d[name] = nc.dram_tensor(
                "brow_" + name, [1, bias_rows[name].shape[0]], BF16,
                kind="ExternalInput",
            ).ap()
    Yd = nc.dram_tensor("y", [TQ, D], BF16, kind="ExternalOutput").ap()

    Qt = Qd.rearrange("(t p) d -> t p d", p=P)
    Kt = Kd.rearrange("(t p) d -> t p d", p=P)
    Vt = Vd.rearrange("(t p) d -> t p d", p=P)
    Yt = Yd.rearrange("(t p) d -> t p d", p=P)
    # weight DRAM views: [P, chunk, cols]
    wq_v = wq_d.rearrange("(c p) n -> p c n", p=P)
    wk_v = wk_d.rearrange("(c p) n -> p c n", p=P)
    wv_v = wv_d.rearrange("(c p) n -> p c n", p=P)
    wo_v = wo_d.rearrange("(c p) n -> p c n", p=P)
    w1_v = w1_d.rearrange("p (m e q) -> p m e q", m=MC, e=DC)  # [P,MC,8,P]
    w2_v = w2_d.rearrange("p (e n) -> p e n", e=MC)

    with tile.TileContext(nc) as tc:
        with tc.tile_pool(name="singles", bufs=1) as singles:
            # persistent weights (wk/wv live in a scoped pool around phase 1
            # and their SBUF is recycled for the phase-2 pools)
            wq_sb = singles.tile([P, DC, INNER], BF16)
            wo_sb = singles.tile([P, IC, D], BF16)
            w1_sb = singles.tile([P, MC, DC, P], FP8)
            w2_sb = singles.tile([P, MC, D], FP8)
            bff1_sb = singles.tile([P, MC], F32)
            eps_tile = singles.tile([P, 1], F32)
            nc.vector.memset(eps_tile[:], EPS)
            ones_row = singles.tile([1, P], BF16)
            nc.vector.memset(ones_row[:], 1.0)
            brow_sb = {}
            for name, ap in brow_d.items():
                t = singles.tile([1, ap.shape[1]], BF16, tag="brow_" + name)
                nc.gpsimd.dma_start(t[:], ap[:])
                brow_sb[name] = t
            # residual / LN2 source (bf16: feeds LN stats + final residual)
            x_sb = singles.tile([P, NQT, D], BF16)
            # head-pair attention matrices: M_sb[:, pr, :] is
            # blockdiag(M_2pr, M_2pr+1); off-diagonal junk stays zero
            M_sb = singles.tile([P, IC, P], BF16)
            nc.vector.memset(M_sb[:], 0.0)
            # feature-major fp8 LN2(x) for the whole 1024-token slab
            xnT8_sb = singles.tile([P, DC, TQ], FP8)

            # ---------------- Phase 1: K/V -> M_h ----------------
            with (
                tc.tile_pool(name="kvw", bufs=1) as kvw,
                tc.tile_pool(name="kv_io", bufs=6) as kv_io,
                tc.tile_pool(name="kv_mid", bufs=3) as kv_mid,
                tc.tile_pool(name="kv_stats", bufs=4) as kv_stats,
                tc.tile_pool(name="kv_ps", bufs=6, space="PSUM") as kv_ps,
                tc.tile_pool(name="m_ps", bufs=1, space="PSUM") as m_ps_pool,
            ):
                # wk/wv first on the Pool queue (chunked into separate tiles:
                # tile 0's projection starts after one chunk and later chunk
                # writes can't alias earlier reads), then later-phase weights.
                wk_sb = [kvw.tile([P, INNER], BF16, name=f"wk{c}") for c in range(DC)]
                wv_sb = [kvw.tile([P, INNER], BF16, name=f"wv{c}") for c in range(DC)]
                for c in range(DC):
                    nc.gpsimd.dma_start(wk_sb[c][:], wk_v[:, c, :])
                for c in range(DC):
                    nc.gpsimd.dma_start(wv_sb[c][:], wv_v[:, c, :])
                nc.gpsimd.dma_start(wq_sb[:], wq_v[:])
                nc.gpsimd.dma_start(wo_sb[:], wo_v[:])
                for mg in range(4):
                    nc.gpsimd.dma_start(
                        w1_sb[:, mg * 8 : (mg + 1) * 8], w1_v[:, mg * 8 : (mg + 1) * 8]
                    )
                for mg in range(4):
                    nc.gpsimd.dma_start(
                        w2_sb[:, mg * 8 : (mg + 1) * 8], w2_v[:, mg * 8 : (mg + 1) * 8]
                    )
                nc.gpsimd.dma_start(bff1_sb[:], bff1_d[:])
                M_ps = m_ps_pool.tile([P, IC, P], F32)

                def m_accum(t, kn_bf, v_bf):
                    # M_h accumulation: M[h] += kn_h^T @ v_h
                    # Heads are processed in pairs: one [128,128] matmul per
                    # pair computes blockdiag(M_2pr, M_2pr+1) plus junk
                    # off-diagonal blocks (discarded at evac). start=True
                    # zeroes the whole 2KB PSUM zero-region (= 4 pair blocks),
                    # so only the first pair per region starts the group and
                    # only the last stops it.  Called one tile late (software
                    # pipelining) so the kn/v evac chains are never on the
                    # TensorE critical path.
                    kn_flat = kn_bf.rearrange("p h f -> p (h f)")
                    for pr in range(IC):
                        nc.tensor.matmul(
                            M_ps[:, pr, :],
                            kn_flat[:, pr * P : (pr + 1) * P],
                            v_bf[:, pr * P : (pr + 1) * P],
                            start=(t == 0 and pr % 4 == 0),
                            stop=(t == NKT - 1 and pr % 4 == 3),
                            skip_group_check=True,
                        )

                prev_kv = None
                for t in range(NKT):
                    kn_bf = None
                    v_bf = None
                    for which in ("k", "v"):
                        src = Kt[t] if which == "k" else Vt[t]
                        w_sb = wk_sb if which == "k" else wv_sb
                        x_in = kv_io.tile([P, D], BF16, tag="kv_in")
                        nc.sync.dma_start(x_in[:], src[:])
                        # transpose RAW tile; LN mean is folded into Wc and
                        # rstd either cancels (K) or scales the evac (V)
                        xnT = kv_mid.tile([P, DC, P], BF16, tag="kv_xnT")
                        for c in range(DC):
                            nc.sync.dma_start(
                                xnT[:, c, :], x_in[:, c * P : (c + 1) * P],
                                transpose=True,
                            )
                        if which == "v":
                            rs_v = _var_rstd_ops(nc, kv_stats, x_in, eps_tile)
                        pss = []
                        for g in range(2):
                            ps = kv_ps.tile([P, 512], F32, tag="kv_proj")
                            pss.append(ps)
                        for c in range(DC):
                            for g in range(2):
                                nc.tensor.matmul(
                                    pss[g][:],
                                    xnT[:, c, :],
                                    w_sb[c][:, g * 512 : (g + 1) * 512],
                                    start=(c == 0),
                                    stop=(c == DC - 1),
                                )
                        if which == "v":
                            # v = rstd * (x @ wv_c)
                            v_bf = kv_mid.tile([P, INNER], BF16, tag="v_bf")
                            for g in range(2):
                                nc.scalar.activation(
                                    out=v_bf[:, g * 512 : (g + 1) * 512],
                                    in_=pss[g][:],
                                    func=AF.Copy,
                                    scale=rs_v[:],
                                )
                        else:
                            # l2-normalize per head (rstd cancels)
                            kn_bf = kv_mid.tile([P, H, DH], BF16, tag="kn_bf")
                            for g in range(2):
                                sq = kv_mid.tile([P, 512], F32, tag="kv_sq")
                                nc.scalar.activation(
                                    out=sq[:], in_=pss[g][:], func=AF.Square
                                )
                                ss = kv_stats.tile([P, 8, 1], F32, tag="l2_ss")
                                nc.vector.reduce_sum(
                                    out=ss[:],
                                    in_=sq.rearrange("p (h f) -> p h f", h=8),
                                    axis=mybir.AxisListType.X,
                                )
                                rn = kv_stats.tile([P, 8, 1], F32, tag="l2_rn")
                                nc.scalar.activation(
                                    out=rn[:], in_=ss[:], func=AF.Sqrt
                                )
                                nc.vector.tensor_scalar_max(
                                    out=rn[:], in0=rn[:], scalar1=1e-12
                                )
                                nc.vector.reciprocal(out=rn[:], in_=rn[:])
                                nc.vector.tensor_tensor(
                                    out=kn_bf[:, g * 8 : (g + 1) * 8, :],
                                    in0=pss[g].rearrange("p (h f) -> p h f", h=8),
                                    in1=rn.to_broadcast([P, 8, DH]),
                                    op=ALU.mult,
                                )
                    if prev_kv is not None:
                        m_accum(t - 1, *prev_kv)
                    prev_kv = (kn_bf, v_bf)
                m_accum(NKT - 1, *prev_kv)
                for po in (0, 64):
                    nc.scalar.activation(
                        out=M_sb[po : po + 64, :, po : po + 64],
                        in_=M_ps[po : po + 64, :, po : po + 64],
                        func=AF.Copy,
                    )

            # ---------------- Phase 2: Q -> attn -> x -> LN2 ----------------
            with (
                tc.tile_pool(name="q_io", bufs=NQT) as q_io,
                tc.tile_pool(name="q_mid", bufs=2) as q_mid,
                tc.tile_pool(name="q_qn", bufs=NQT) as q_qn,
                tc.tile_pool(name="q_at", bufs=NQT) as q_at,
                tc.tile_pool(name="q_stats", bufs=4) as q_stats,
                tc.tile_pool(name="f_mid", bufs=2) as f_mid,
                tc.tile_pool(name="f_stats", bufs=4) as f_stats,
                tc.tile_pool(name="q_ps", bufs=2, space="PSUM") as q_ps,
                tc.tile_pool(name="x_ps", bufs=2, space="PSUM") as x_ps,
                tc.tile_pool(name="at_ps", bufs=2, space="PSUM") as at_ps,
            ):
                # --- qn prepass: all tiles ---
                q_ins = []
                qnT2s = []
                for t in range(NQT):
                    q_in = q_io.tile([P, D], BF16, tag="q_in", name=f"q_in{t}")
                    nc.sync.dma_start(q_in[:], Qt[t][:])
                    q_ins.append(q_in)
                    qnT = q_mid.tile([P, DC, P], BF16, tag="q_xnT")
                    for c in range(DC):
                        nc.sync.dma_start(
                            qnT[:, c, :], q_in[:, c * P : (c + 1) * P], transpose=True
                        )
                    pss = []
                    for g in range(2):
                        ps = q_ps.tile([P, 512], F32, tag="q_proj")
                        pss.append(ps)
                    for c in range(DC):
                        for g in range(2):
                            nc.tensor.matmul(
                                pss[g][:],
                                qnT[:, c, :],
                                wq_sb[:, c, g * 512 : (g + 1) * 512],
                                start=(c == 0),
                                stop=(c == DC - 1),
                            )
                    # l2-normalize q per head (rstd cancels) -> qn bf16
                    qn_bf = q_mid.tile([P, H, DH], BF16, tag="qn_bf")
                    for g in range(2):
                        sq = q_mid.tile([P, 512], F32, tag="q_sq")
                        nc.scalar.activation(out=sq[:], in_=pss[g][:], func=AF.Square)
                        ss = q_stats.tile([P, 8, 1], F32, tag="ql2_ss")
                        nc.vector.reduce_sum(
                            out=ss[:],
                            in_=sq.rearrange("p (h f) -> p h f", h=8),
                            axis=mybir.AxisListType.X,
                        )
                        rn = q_stats.tile([P, 8, 1], F32, tag="ql2_rn")
                        nc.scalar.activation(out=rn[:], in_=ss[:], func=AF.Sqrt)
                        nc.vector.tensor_scalar_max(out=rn[:], in0=rn[:], scalar1=1e-12)
                        nc.vector.reciprocal(out=rn[:], in_=rn[:])
                        nc.vector.tensor_tensor(
                            out=qn_bf[:, g * 8 : (g + 1) * 8, :],
                            in0=pss[g].rearrange("p (h f) -> p h f", h=8),
                            in1=rn.to_broadcast([P, 8, DH]),
                            op=ALU.mult,
                        )
                    # transpose qn -> [INNER, tok] feature-major
                    qn_flat = qn_bf.rearrange("p h f -> p (h f)")
                    qnT2 = q_qn.tile([P, IC, P], BF16, tag="qnT2", name=f"qnT2_{t}")
                    for c in range(IC):
                        nc.sync.dma_start(
                            qnT2[:, c, :], qn_flat[:, c * P : (c + 1) * P],
                            transpose=True,
                        )
                    qnT2s.append(qnT2)
                # --- attn + wo + residual + LN2(FFN stage A) per tile ---
                aT_bfs = []
                for t in range(NQT):
                    # attn^T[h] = M_h^T @ qn_h^T  -> [INNER, tok] chunks
                    a_ps = at_ps.tile([P, IC, P], F32, tag="attn_ps")
                    for pr in range(IC):
                        nc.tensor.matmul(
                            a_ps[:, pr, :],
                            M_sb[:, pr, :],
                            qnT2s[t][:, pr, :],
                            start=True,
                            stop=True,
                            skip_group_check=True,
                        )
                    aT_bf = q_at.tile([P, IC, P], BF16, tag="aT_bf")
                    nc.scalar.activation(out=aT_bf[:], in_=a_ps[:], func=AF.Copy)
                    aT_bfs.append(aT_bf)
                for t in range(NQT):
                    aT_bf = aT_bfs[t]
                    # x = Q + attn @ wo (+bo)
                    xps = []
                    for g in range(2):
                        ps = x_ps.tile([P, 512], F32, tag="x_proj")
                        xps.append(ps)
                    for c in range(IC):
                        for g in range(2):
                            nc.tensor.matmul(
                                xps[g][:],
                                aT_bf[:, c, :],
                                wo_sb[:, c, g * 512 : (g + 1) * 512],
                                start=(c == 0),
                                stop=(c == IC - 1) and ("bo" not in brow_sb),
                            )
                    for g in range(2):
                        if "bo" in brow_sb:
                            nc.tensor.matmul(
                                xps[g][:],
                                ones_row[:],
                                brow_sb["bo"][:, g * 512 : (g + 1) * 512],
                                start=False,
                                stop=True,
                                skip_group_check=True,
                            )
                        # x = 1.0*xps + q_in  (fused DVE op; PSUM in0)
                        nc.vector.scalar_tensor_tensor(
                            out=x_sb[:, t, g * 512 : (g + 1) * 512],
                            in0=xps[g][:],
                            scalar=1.0,
                            in1=q_ins[t][:, g * 512 : (g + 1) * 512],
                            op0=ALU.mult,
                            op1=ALU.add,
                        )
                    # FFN stage A for this tile: LN2 -> transpose -> fp8 cast.
                    # Emitted here so it runs in the attention-phase slack.
                    rs, nmu = _ln_stats_ops(nc, f_stats, x_sb[:, t, :], P, D, eps_tile)
                    xn = f_mid.tile([P, D], BF16, tag="f_std")
                    nc.scalar.activation(
                        out=xn[:],
                        in_=x_sb[:, t, :],
                        func=AF.Identity,
                        bias=nmu[:],
                        scale=rs[:],
                    )
                    xnT = f_mid.tile([P, DC, P], BF16, tag="f_xnT")
                    for c in range(DC):
                        nc.sync.dma_start(
                            xnT[:, c, :], xn[:, c * P : (c + 1) * P], transpose=True
                        )
                    # cast the tile's transposed slab to fp8 (Pool engine)
                    nc.gpsimd.tensor_copy(
                        out=xnT8_sb[:, :, t * P : (t + 1) * P],
                        in_=xnT[:],
                    )

            # ---------------- Phase 3: FFN (fp8 DoubleRow) ----------------
            # h^T = gelu(w1^T xn^T) in fp8, then y = x + h @ w2 / W2S
            # token-major.  DoubleRow packs two 128-deep chunks per matmul.
            with (
                tc.tile_pool(name="f_h", bufs=1) as f_h,
                tc.tile_pool(name="f_out", bufs=3) as f_out,
                tc.tile_pool(name="h_ps", bufs=2, space="PSUM") as h_ps,
                tc.tile_pool(name="y_ps", bufs=4, space="PSUM") as y_ps,
            ):
                GT = 8  # token tiles per group (single group: the w2 matmuls
                # drain while the Activation engine finishes the gelu backlog)
                for g3 in range(NQT // GT):
                    tok0 = g3 * GT * P
                    h8 = f_h.tile([P, MC, GT * P], FP8, tag="h8")
                    for m in range(MC):
                        # one matmul per (contraction pair, 512-col half):
                        # a single matmul output may not exceed one PSUM bank
                        hp = h_ps.tile([P, GT * P // 512, 512], F32, tag="h_ps_t")
                        for j in range(DC // 2):
                            for g4 in range(GT * P // 512):
                                nc.tensor.matmul(
                                    hp[:, g4, :],
                                    w1_sb[:, m, 2 * j : 2 * j + 2, :],
                                    xnT8_sb[:, 2 * j : 2 * j + 2,
                                            tok0 + g4 * 512 : tok0 + (g4 + 1) * 512],
                                    start=(j == 0),
                                    stop=(j == DC // 2 - 1),
                                    perf_mode=DR,
                                )
                        # h = gelu(psum / W1S + b1row), straight to fp8
                        nc.scalar.activation(
                            out=h8[:, m, :],
                            in_=hp.rearrange("p a b -> p (a b)"),
                            func=AF.Gelu,
                            bias=bff1_sb[:, m : m + 1],
                            scale=1.0 / W1S,
                        )
                    for tt in range(GT):
                        t = g3 * GT + tt
                        for g in range(2):
                            yps = y_ps.tile([P, 512], F32, tag="y_ps_t")
                            for j2 in range(MC // 2):
                                nc.tensor.matmul(
                                    yps[:],
                                    h8[:, 2 * j2 : 2 * j2 + 2, tt * P : (tt + 1) * P],
                                    w2_sb[:, 2 * j2 : 2 * j2 + 2, g * 512 : (g + 1) * 512],
                                    start=(j2 == 0),
                                    stop=(j2 == MC // 2 - 1) and ("b2" not in brow_sb),
                                    perf_mode=DR,
                                )
                            if "b2" in brow_sb:
                                nc.tensor.matmul(
                                    yps[:],
                                    ones_row[:],
                                    brow_sb["b2"][:, g * 512 : (g + 1) * 512],
                                    start=False,
                                    stop=True,
                                    skip_group_check=True,
                                )
                            y_out = f_out.tile([P, 512], BF16, tag="y_out")
                            # y = yps / W2S + x  (single fused DVE op)
                            nc.vector.scalar_tensor_tensor(
                                out=y_out[:],
                                in0=yps[:],
                                scalar=1.0 / W2S,
                                in1=x_sb[:, t, g * 512 : (g + 1) * 512],
                                op0=ALU.mult,
                                op1=ALU.add,
                            )
                            nc.sync.dma_start(
                                Yt[t][:, g * 512 : (g + 1) * 512], y_out[:]
                            )

    nc.compile()
    return nc


def prep_inputs(inputs):
    """Host-side shard + weight folding. Returns (in_maps, bias_rows)."""
    f32 = np.float32
    bf = ml_dtypes.bfloat16
    f8 = ml_dtypes.float8_e4m3fn
    g1 = np.asarray(inputs["ln1_g"], f32)
    b1ln = np.asarray(inputs["ln1_b"], f32)
    g2 = np.asarray(inputs["ln2_g"], f32)
    b2ln = np.asarray(inputs["ln2_b"], f32)
    wq = np.asarray(inputs["wq"], f32)
    wk = np.asarray(inputs["wk"], f32)
    wv = np.asarray(inputs["wv"], f32)
    wo = np.asarray(inputs["wo"], f32)
    w1 = np.asarray(inputs["w1"], f32)
    w2 = np.asarray(inputs["w2"], f32)

    bias_rows = {
        "bq": (b1ln @ wq).astype(f32),
        "bk": (b1ln @ wk).astype(f32),
        "bv": (b1ln @ wv).astype(f32),
        "bo": np.asarray(inputs["bo"], f32),
        # b2 joins the W2S-scaled yps accumulation group
        "b2": (np.asarray(inputs["b2"], f32) * W2S),
    }
    bff1 = (b2ln @ w1 + np.asarray(inputs["b1"], f32)).astype(f32)
    bff1_tile = np.ascontiguousarray(bff1.reshape(MC, P).T)  # [P, MC]

    def mean_fold(Wg):
        # LN(x) @ Wg == rstd(x) * (x @ Wc):  Wc = Wg - (1/D) * 1 (x) colsum(Wg)
        return Wg - Wg.sum(axis=0, keepdims=True) / Wg.shape[0]

    wq_b = np.ascontiguousarray(mean_fold(g1[:, None] * wq).astype(bf))
    wk_b = np.ascontiguousarray(mean_fold(g1[:, None] * wk).astype(bf))
    wv_b = np.ascontiguousarray(mean_fold(g1[:, None] * wv).astype(bf))
    wo_b = np.ascontiguousarray(wo.astype(bf))
    # fp8 packs (see build_nc for index meaning); LN2's mean stays on-chip
    # (the xn path) so w1 keeps only the gamma fold.
    w1g = (g2[:, None] * w1) * W1S
    w1_b = np.ascontiguousarray(
        w1g.reshape(DC, P, MC, P).transpose(1, 2, 0, 3).reshape(P, MC * DC * P)
    ).astype(f8)
    w2s = w2 * W2S
    w2_b = np.ascontiguousarray(
        w2s.reshape(MC, P, D).transpose(1, 0, 2).reshape(P, MC * D)
    ).astype(f8)

    Q = np.asarray(inputs["Q"], f32)
    K = np.asarray(inputs["K"], f32)
    V = np.asarray(inputs["V"], f32)

    in_maps = []
    for c in range(NCORES):
        b = c // 2
        r0 = (c % 2) * TQ
        m = {
            "q_tok": np.ascontiguousarray(Q[b, r0 : r0 + TQ].astype(bf)),
            "k_tok": np.ascontiguousarray(K[b].astype(bf)),
            "v_tok": np.ascontiguousarray(V[b].astype(bf)),
            "wq": wq_b,
            "wk": wk_b,
            "wv": wv_b,
            "wo": wo_b,
            "w1p": w1_b,
            "w2p": w2_b,
            "bff1": bff1_tile,
        }
        for name, row in bias_rows.items():
            if name in ("bo", "b2") and np.any(row):
                m["brow_" + name] = row[None, :].astype(bf)
        in_maps.append(m)
    return in_maps, bias_rows


_NC_CACHE = {}


def kernel(**inputs) -> np.ndarray:
    from concourse.bass_utils import run_bass_kernel_spmd

    in_maps, bias_rows = prep_inputs(inputs)
    bias_key = tuple(sorted(n for n, r in bias_rows.items() if np.any(r)))
    if bias_key not in _NC_CACHE:
        _NC_CACHE[bias_key] = build_nc(bias_rows)
    nc = _NC_CACHE[bias_key]
    res = run_bass_kernel_spmd(nc, in_maps, core_ids=list(range(NCORES)))
    out = np.empty((B, N, D), np.float32)
    for c in range(NCORES):
        b = c // 2
        r0 = (c % 2) * TQ
        out[b, r0 : r0 + TQ] = res.results[c]["y"]
    return out
